# revision 6
# baseline (speedup 1.0000x reference)
"""Trainium2 Bass kernel for nn_CTMBlock (B=8, T=2048, D=256, 3 ticks).

Sharding: data-parallel over B across 8 NeuronCores (1 batch element per
core), d x d weights replicated, pred_loss / eff_p reduced on host.

Per-core algorithm (flash-style, no [T,T] tensor ever touches HBM):
  tick:
    x1   = z @ in_w.T + in_b            (token-major, PE; bias via K=1 mm)
    LN   via bn_stats/bn_aggr           (DVE), h = (x1-mu)*rstd
           ln_g folded into q/k/v weights, ln_b folded into their biases
    h_T  = transpose(h)                 (PE transposes)
    Q_T, K_T feature-major; V token-major with bias via K=1 ones-matmul
    pass1: S = Q K^T /sqrt(D) per 128-row block (PE) -> exp (ACT, fused
           row-sum via accum_out) -> row max (DVE).  row_max(softmax) =
           max(exp)/sum(exp).  conf/eff_p scalar chain, Q_T *= eff_p.
    pass2: S_T = K Q_p2^T s-major (PE) -> exp (ACT) = P_T; ctx_tok
           accumulated as P_T^T @ V; softmax denominator as P_T^T @ 1;
           normalize on eviction (per-partition scalar), transpose ctx
           back to feature-major (PE).
    target = o_w @ ctx + o_b; dz = (target - z) * DT/(tau+1e-6)
    z_pred = p2(relu(p1(z))); loss = sum((z_pred - z_next)^2)
"""

import sys

for _p in ("/opt/trn_rl_repo", "/root/.axon_site/_ro/trn_rl_repo"):
    if _p not in sys.path:
        sys.path.insert(0, _p)

import math

import numpy as np

import concourse.bass as bass
import concourse.bacc as bacc
import concourse.tile as tile
from concourse import mybir
from concourse.bass_utils import run_bass_kernel_spmd

F32 = mybir.dt.float32
AX = mybir.AxisListType
ALU = mybir.AluOpType
ACT = mybir.ActivationFunctionType

B = 8
T = 2048
D = 256
P = 128
KB = D // P            # 2 k-blocks of the feature dim
NT = T // P            # 16 token tiles
CH = 512               # fp32 matmul max moving free dim
NCH = T // CH          # 4 chunks
TICKS = 3
DT_STEP = 1.0
P_MIN, P_MAX = 2.0, 4.0
LN_EPS = 1e-5
TAU_EPS = 1e-6
INV_SQRT_D = 1.0 / math.sqrt(D)
UNIFORM = 1.0 / T
CONF_SCALE = 1.0 / (1.0 - UNIFORM + 1e-6)

_NC_CACHE = {}


def _build_nc():
    """Build the per-core Bass program (identical on all 8 cores)."""
    nc = bacc.Bacc("TRN2", target_bir_lowering=False, debug=False,
                   num_devices=B)

    io = {}

    def din(name, shape):
        io[name] = nc.dram_tensor(name, shape, F32,
                                  kind="ExternalInput").ap()

    for w in ("w_in", "w_q", "w_k", "w_v", "w_o", "w_p1", "w_p2"):
        din(w, [D, D])
    din("z0", [D, T])
    din("b_in_r", [1, D])
    din("b_v_r", [1, D])
    for b in ("b_q", "b_k", "b_o", "b_p1", "b_p2", "dtrt"):
        din(b, [D, 1])
    din("ident", [P, P])
    io["z_out"] = nc.dram_tensor("z_out", [D, T], F32,
                                 kind="ExternalOutput").ap()
    io["stats_out"] = nc.dram_tensor("stats", [1, 8], F32,
                                     kind="ExternalOutput").ap()

    with tile.TileContext(nc) as tc:
        _emit(nc, tc, io)
    nc.compile()
    return nc


def _emit(nc, tc, io):
    ctxs = []

    def pool(name, bufs, space="SBUF"):
        p = tc.tile_pool(name=name, bufs=bufs, space=space)
        ctxs.append(p)
        return p.__enter__()

    consts = pool("consts", 1)
    zstate = pool("zstate", 4)     # 2 tiles per generation, ping-pong
    actp = pool("actp", 1)         # hT/cT, qT, kT feature-major [128,2048]
    med = pool("med", 3)           # per-chunk scratch
    ctxsb = pool("ctxsb", 16)      # normalized ctx tiles, all 16 alive
    small = pool("small", 4)       # [128,<=16] stats
    exp1p = pool("exp1p", 2)       # pass1 exp rows [128,2048]
    ptp = pool("ptp", 3)           # pass2 P_T tiles [128,512]
    # PSUM: 8 banks total, statically partitioned:
    ps_work = pool("ps_work", 2, "PSUM")   # tag work: [128,1024] -> 4 banks
    ps_ctx = pool("ps_ctx", 1, "PSUM")     # [128,4,512] -> 4 banks
    # NOTE: each matmul accumulation group must own its full 2KB PSUM
    # zero-region (start=True lazily zeroes the whole region), hence the
    # 512-f32 stride between the four ctx accumulators.

    def work(shape, name):
        return ps_work.tile(shape, F32, tag="work", name=name,
                            padded_shape=[P, 1024])

    dma = nc.sync

    # ---- constants ----
    def load_pair(name, shape):
        tiles = []
        for kb in range(KB):
            t = consts.tile(shape, F32, tag=f"{name}{kb}", name=f"{name}{kb}")
            dma.dma_start(out=t, in_=io[name][kb * P:(kb + 1) * P, :])
            tiles.append(t)
        return tiles

    w_sb = {w: load_pair(w, [P, D])
            for w in ("w_in", "w_q", "w_k", "w_v", "w_o", "w_p1", "w_p2")}
    col_sb = {b: load_pair(b, [P, 1])
              for b in ("b_q", "b_k", "b_o", "b_p1", "b_p2", "dtrt")}

    b_in_sb = consts.tile([1, D], F32, tag="b_in_r", name="b_in_r")
    dma.dma_start(out=b_in_sb, in_=io["b_in_r"])
    b_v_sb = consts.tile([1, D], F32, tag="b_v_r", name="b_v_r")
    dma.dma_start(out=b_v_sb, in_=io["b_v_r"])
    ident_sb = consts.tile([P, P], F32, tag="ident", name="ident")
    dma.dma_start(out=ident_sb, in_=io["ident"])

    ones_col = consts.tile([P, 1], F32, tag="ones_col", name="ones_col")
    nc.vector.memset(ones_col, 1.0)
    ones_row = consts.tile([1, P], F32, tag="ones_row", name="ones_row")
    nc.vector.memset(ones_row, 1.0)
    stats_sb = consts.tile([1, 8], F32, tag="stats_sb", name="stats_sb")
    nc.vector.memset(stats_sb, 0.0)
    c_eps = consts.tile([P, 1], F32, tag="c_eps", name="c_eps")
    nc.vector.memset(c_eps, LN_EPS)
    c_confb = consts.tile([P, 1], F32, tag="c_confb", name="c_confb")
    nc.vector.memset(c_confb, -UNIFORM * CONF_SCALE)
    c_pmin = consts.tile([P, 1], F32, tag="c_pmin", name="c_pmin")
    nc.vector.memset(c_pmin, P_MIN)

    # V tiles persist (token-major, col D is the all-ones denominator col)
    v_sb = [consts.tile([P, D + 1], F32, tag=f"v{i}", name=f"v{i}")
            for i in range(NT)]
    for i in range(NT):
        nc.vector.memset(v_sb[i][:, D:D + 1], 1.0)

    # ---- z state ----
    z_cur = [zstate.tile([P, T], F32, tag="z", name="z") for _ in range(KB)]
    for kb in range(KB):
        dma.dma_start(out=z_cur[kb], in_=io["z0"][kb * P:(kb + 1) * P, :])

    for tick in range(TICKS):
        # ======== phase A: x1 (token-major) + LN + transpose to h_T ========
        h_T = [actp.tile([P, T], F32, tag=f"hT{kb}", name=f"hT{kb}")
               for kb in range(KB)]
        for g4 in range(NT // 4):
            htoks = []
            for j in range(4):
                tt = g4 * 4 + j
                x1p = work([P, D], "x1p")
                for kb in range(KB):
                    nc.tensor.matmul(x1p, z_cur[kb][:, tt * P:(tt + 1) * P],
                                     w_sb["w_in"][kb], start=(kb == 0),
                                     stop=False)
                nc.tensor.matmul(x1p, ones_row, b_in_sb, start=False,
                                 stop=True)
                x1sb = med.tile([P, D], F32, tag="x1sb", name="x1sb",
                                bufs=4)
                if tt % 2 == 0:
                    nc.scalar.copy(x1sb, x1p)
                else:
                    nc.vector.tensor_copy(x1sb, x1p)
                bn6 = small.tile([P, 6], F32, tag="bn6", name="bn6")
                nc.vector.bn_stats(bn6, x1sb)
                mv = small.tile([P, 2], F32, tag="mv", name="mv")
                nc.vector.bn_aggr(mv, bn6)
                rstd = small.tile([P, 1], F32, tag="rstd", name="rstd")
                nc.scalar.activation(rstd, mv[:, 1:2], ACT.Sqrt, bias=c_eps)
                nc.vector.reciprocal(rstd, rstd)
                htok = med.tile([P, D], F32, tag="htok", name="htok",
                                bufs=8)
                nc.vector.tensor_scalar(htok, x1sb, mv[:, 0:1], rstd,
                                        op0=ALU.subtract, op1=ALU.mult)
                htoks.append(htok)
            for db in range(KB):
                tr = work([P, CH], "trA")
                for j in range(4):
                    nc.tensor.transpose(tr[:, j * P:(j + 1) * P],
                                        htoks[j][:, db * P:(db + 1) * P],
                                        ident_sb)
                dst = h_T[db][:, g4 * CH:(g4 + 1) * CH]
                if db == 0:
                    nc.scalar.copy(dst, tr)
                else:
                    nc.vector.tensor_copy(dst, tr)

        # ======== phase B: Q, K feature-major; V token-major ========
        q_T = [actp.tile([P, T], F32, tag=f"qT{kb}", name=f"qT{kb}")
               for kb in range(KB)]
        k_T = [actp.tile([P, T], F32, tag=f"kT{kb}", name=f"kT{kb}")
               for kb in range(KB)]
        for db in range(KB):
            for ch in range(NCH):
                qp = work([P, CH], "qp")
                for kb in range(KB):
                    nc.tensor.matmul(qp, w_sb["w_q"][kb][:, db * P:(db + 1) * P],
                                     h_T[kb][:, ch * CH:(ch + 1) * CH],
                                     start=(kb == 0), stop=(kb == KB - 1))
                nc.scalar.activation(q_T[db][:, ch * CH:(ch + 1) * CH], qp,
                                     ACT.Identity, bias=col_sb["b_q"][db],
                                     scale=INV_SQRT_D)
                kp = work([P, CH], "kp")
                for kb in range(KB):
                    nc.tensor.matmul(kp, w_sb["w_k"][kb][:, db * P:(db + 1) * P],
                                     h_T[kb][:, ch * CH:(ch + 1) * CH],
                                     start=(kb == 0), stop=(kb == KB - 1))
                nc.vector.tensor_scalar_add(k_T[db][:, ch * CH:(ch + 1) * CH],
                                            kp, col_sb["b_k"][db])
        for st in range(NT):
            vp = work([P, D], "vp")
            for kb in range(KB):
                nc.tensor.matmul(vp, h_T[kb][:, st * P:(st + 1) * P],
                                 w_sb["w_v"][kb], start=(kb == 0), stop=False)
            nc.tensor.matmul(vp, ones_row, b_v_sb, start=False, stop=True)
            if st % 2 == 0:
                nc.scalar.copy(v_sb[st][:, 0:D], vp)
            else:
                nc.vector.tensor_copy(v_sb[st][:, 0:D], vp)

        # ======== phase C: pass 1 (plain softmax row stats) ========
        rma = small.tile([P, NT], F32, tag="rma", name="rma")
        for tt in range(NT):
            ex = exp1p.tile([P, T], F32, tag="ex", name="ex")
            ses = []
            for hf in range(2):
                sp = work([P, T // 2], "sp")
                for kb in range(KB):
                    for c2 in range(2):
                        sl = slice(c2 * CH, (c2 + 1) * CH)
                        nc.tensor.matmul(
                            sp[:, sl], q_T[kb][:, tt * P:(tt + 1) * P],
                            k_T[kb][:, hf * (T // 2) + c2 * CH:
                                    hf * (T // 2) + (c2 + 1) * CH],
                            start=(kb == 0), stop=(kb == KB - 1))
                se = small.tile([P, 1], F32, tag=f"se{hf}", name=f"se{hf}")
                nc.scalar.activation(ex[:, hf * (T // 2):(hf + 1) * (T // 2)],
                                     sp, ACT.Exp, accum_out=se)
                ses.append(se)
            sesum = small.tile([P, 1], F32, tag="sesum", name="sesum")
            nc.vector.tensor_tensor(sesum, ses[0], ses[1], op=ALU.add)
            mx = small.tile([P, 1], F32, tag="mx", name="mx")
            nc.vector.reduce_max(mx, ex, axis=AX.X)
            rcp = small.tile([P, 1], F32, tag="rcp", name="rcp")
            nc.vector.reciprocal(rcp, sesum)
            nc.vector.tensor_tensor(rma[:, tt:tt + 1], mx, rcp, op=ALU.mult)
        rms = small.tile([P, 1], F32, tag="rms", name="rms")
        nc.vector.reduce_sum(rms, rma, axis=AX.X)
        tot = work([1, 1], "tot")
        nc.tensor.matmul(tot, ones_col, rms, start=True, stop=True)
        conf = small.tile([1, 1], F32, tag="conf", name="conf")
        nc.scalar.activation(conf, tot, ACT.Identity,
                             bias=c_confb[:1], scale=CONF_SCALE / T)
        nc.vector.tensor_scalar(conf, conf, 0.0, 1.0, op0=ALU.max,
                                op1=ALU.min)
        effp = small.tile([1, 1], F32, tag="effp", name="effp")
        nc.scalar.activation(effp, conf, ACT.Identity,
                             bias=c_pmin[:1], scale=P_MAX - P_MIN)
        nc.scalar.copy(stats_sb[:, 3 + tick:4 + tick], effp)
        effb_p = work([P, 1], "effb")
        nc.tensor.matmul(effb_p, ones_row, effp, start=True, stop=True)
        effc = small.tile([P, 1], F32, tag="effc", name="effc")
        nc.scalar.copy(effc, effb_p)
        for kb in range(KB):
            nc.vector.tensor_scalar_mul(q_T[kb], q_T[kb], effc)

        # ======== phase D: pass 2 (sharp softmax @ V) ========
        csb = []
        for g in range(NCH):
            cx = ps_ctx.tile([P, 4, CH], F32, tag="cx", name="cx")
            for st in range(NT):
                stp = work([P, CH], "stp")
                for kb in range(KB):
                    nc.tensor.matmul(stp, k_T[kb][:, st * P:(st + 1) * P],
                                     q_T[kb][:, g * CH:(g + 1) * CH],
                                     start=(kb == 0), stop=(kb == KB - 1))
                pt = ptp.tile([P, CH], F32, tag="pt", name="pt")
                nc.scalar.activation(pt, stp, ACT.Exp)
                for j in range(4):
                    nc.tensor.matmul(cx[:, j, 0:D + 1],
                                     pt[:, j * P:(j + 1) * P],
                                     v_sb[st], start=(st == 0),
                                     stop=(st == NT - 1))
            for j in range(4):
                rc = small.tile([P, 1], F32, tag="rc", name="rc")
                nc.vector.reciprocal(rc, cx[:, j, D:D + 1])
                c = ctxsb.tile([P, D], F32, tag="csb", name="csb")
                nc.vector.tensor_scalar_mul(c, cx[:, j, 0:D], rc)
                csb.append(c)
        ctx_T = [actp.tile([P, T], F32, tag=f"hT{kb}", name=f"cT{kb}")
                 for kb in range(KB)]
        for db in range(KB):
            for g4 in range(NT // 4):
                tr = work([P, CH], "trD")
                for j in range(4):
                    nc.tensor.transpose(tr[:, j * P:(j + 1) * P],
                                        csb[g4 * 4 + j][:, db * P:(db + 1) * P],
                                        ident_sb)
                dst = ctx_T[db][:, g4 * CH:(g4 + 1) * CH]
                if db == 0:
                    nc.scalar.copy(dst, tr)
                else:
                    nc.vector.tensor_copy(dst, tr)

        # ======== phase E: target, z update ========
        z_new = [zstate.tile([P, T], F32, tag="z", name="zn")
                 for _ in range(KB)]
        for db in range(KB):
            for ch in range(NCH):
                sl = slice(ch * CH, (ch + 1) * CH)
                tp = work([P, CH], "tp")
                for kb in range(KB):
                    nc.tensor.matmul(tp, w_sb["w_o"][kb][:, db * P:(db + 1) * P],
                                     ctx_T[kb][:, sl],
                                     start=(kb == 0), stop=(kb == KB - 1))
                dl = med.tile([P, CH], F32, tag="dl", name="dl")
                nc.vector.scalar_tensor_tensor(dl, tp, col_sb["b_o"][db],
                                               z_cur[db][:, sl],
                                               op0=ALU.add, op1=ALU.subtract)
                nc.vector.scalar_tensor_tensor(z_new[db][:, sl], dl,
                                               col_sb["dtrt"][db],
                                               z_cur[db][:, sl],
                                               op0=ALU.mult, op1=ALU.add)

        # ======== phase F: prediction MLP + loss ========
        lq8 = small.tile([P, 2 * NCH], F32, tag="lq8", name="lq8")
        for ch in range(NCH):
            sl = slice(ch * CH, (ch + 1) * CH)
            us = []
            for db in range(KB):
                up = work([P, CH], "up")
                for kb in range(KB):
                    nc.tensor.matmul(up, w_sb["w_p1"][kb][:, db * P:(db + 1) * P],
                                     z_cur[kb][:, sl],
                                     start=(kb == 0), stop=(kb == KB - 1))
                u = med.tile([P, CH], F32, tag=f"u{db}", name=f"u{db}",
                             bufs=2)
                nc.scalar.activation(u, up, ACT.Relu,
                                     bias=col_sb["b_p1"][db])
                us.append(u)
            for db in range(KB):
                zpp = work([P, CH], "zpp")
                for kb in range(KB):
                    nc.tensor.matmul(zpp, w_sb["w_p2"][kb][:, db * P:(db + 1) * P],
                                     us[kb], start=(kb == 0),
                                     stop=(kb == KB - 1))
                zp = med.tile([P, CH], F32, tag="zp", name="zp")
                nc.vector.scalar_tensor_tensor(zp, zpp, col_sb["b_p2"][db],
                                               z_new[db][:, sl],
                                               op0=ALU.add, op1=ALU.subtract)
                nc.scalar.activation(zp, zp, ACT.Square,
                                     accum_out=lq8[:, db * NCH + ch:
                                                   db * NCH + ch + 1])
        lsum = small.tile([P, 1], F32, tag="lsum", name="lsum")
        nc.vector.reduce_sum(lsum, lq8, axis=AX.X)
        ltot = work([1, 1], "ltot")
        nc.tensor.matmul(ltot, ones_col, lsum, start=True, stop=True)
        nc.scalar.copy(stats_sb[:, tick:tick + 1], ltot)

        z_cur = z_new

    for kb in range(KB):
        dma.dma_start(out=io["z_out"][kb * P:(kb + 1) * P, :], in_=z_cur[kb])
    dma.dma_start(out=io["stats_out"], in_=stats_sb)

    for p in reversed(ctxs):
        p.__exit__(None, None, None)


def _host_prep(inputs):
    f = lambda k: np.asarray(inputs[k], dtype=np.float32)
    z = f("z")
    ln_g, ln_b = f("ln_g"), f("ln_b")
    cons = {}
    cons["w_in"] = np.ascontiguousarray(f("in_w").T)
    for nm in ("q", "k", "v"):
        w = f(f"{nm}_w")
        cons[f"w_{nm}"] = np.ascontiguousarray(w.T * ln_g[:, None])
    cons["w_o"] = np.ascontiguousarray(f("o_w").T)
    cons["w_p1"] = np.ascontiguousarray(f("p1_w").T)
    cons["w_p2"] = np.ascontiguousarray(f("p2_w").T)
    cons["b_in_r"] = f("in_b").reshape(1, D)
    cons["b_q"] = ((f("q_b") + f("q_w") @ ln_b) * INV_SQRT_D).reshape(D, 1)
    cons["b_k"] = (f("k_b") + f("k_w") @ ln_b).reshape(D, 1)
    cons["b_v_r"] = (f("v_b") + f("v_w") @ ln_b).reshape(1, D)
    cons["b_o"] = f("o_b").reshape(D, 1)
    cons["b_p1"] = f("p1_b").reshape(D, 1)
    cons["b_p2"] = f("p2_b").reshape(D, 1)
    cons["dtrt"] = (DT_STEP / (np.exp(f("log_tau")) + TAU_EPS)).reshape(D, 1)
    cons["ident"] = np.eye(P, dtype=np.float32)
    cons = {k: np.ascontiguousarray(v, dtype=np.float32)
            for k, v in cons.items()}
    zT = [np.ascontiguousarray(z[b].T) for b in range(B)]
    return zT, cons


def get_nc():
    if "nc" not in _NC_CACHE:
        _NC_CACHE["nc"] = _build_nc()
    return _NC_CACHE["nc"]


def make_in_maps(inputs):
    zT, cons = _host_prep(inputs)
    return [dict(cons, z0=zT[b]) for b in range(B)]


def postprocess(results):
    z_out = np.stack([results[b]["z_out"].T for b in range(B)])
    stats = np.stack([results[b]["stats"][0] for b in range(B)])  # [B, 8]
    pred_loss = np.float32(stats[:, 0:3].mean() / (T * D))
    eff_p = np.float32(stats[:, 3:6].mean())
    return np.ascontiguousarray(z_out, dtype=np.float32), pred_loss, eff_p


def kernel(**inputs):
    nc = get_nc()
    in_maps = make_in_maps(inputs)
    res = run_bass_kernel_spmd(nc, in_maps, list(range(B)))
    return postprocess(res.results)


# revision 13
# speedup vs baseline: 1.0384x; 1.0384x over previous
"""Trainium2 Bass kernel for nn_CTMBlock (B=8, T=2048, D=256, 3 ticks).

Sharding: data-parallel over B across 8 NeuronCores (1 batch element per
core), d x d weights replicated, pred_loss / eff_p reduced on host.

Per-core algorithm (flash-style, no [T,T] tensor ever touches HBM):
  tick:
    x1   = z @ in_w.T + in_b            (token-major, PE; bias via K=1 mm)
    LN   via bn_stats/bn_aggr           (DVE), h = (x1-mu)*rstd
           ln_g folded into q/k/v weights, ln_b folded into their biases
    h_T  = transpose(h)                 (PE transposes)
    Q_T, K_T feature-major; V token-major with bias via K=1 ones-matmul
    pass1: S = Q K^T /sqrt(D) per 128-row block (PE) -> exp (ACT, fused
           row-sum via accum_out) -> row max (DVE).  row_max(softmax) =
           max(exp)/sum(exp).  conf/eff_p scalar chain, Q_T *= eff_p.
    pass2: S_T = K Q_p2^T s-major (PE) -> exp (ACT) = P_T; ctx_tok
           accumulated as P_T^T @ V; softmax denominator as P_T^T @ 1;
           normalize on eviction (per-partition scalar), transpose ctx
           back to feature-major (PE).
    target = o_w @ ctx + o_b; dz = (target - z) * DT/(tau+1e-6)
    z_pred = p2(relu(p1(z))); loss = sum((z_pred - z_next)^2)
"""

import sys

for _p in ("/opt/trn_rl_repo", "/root/.axon_site/_ro/trn_rl_repo"):
    if _p not in sys.path:
        sys.path.insert(0, _p)

import math

import numpy as np

import concourse.bass as bass
import concourse.bacc as bacc
import concourse.tile as tile
from concourse import mybir
from concourse.bass_utils import run_bass_kernel_spmd

F32 = mybir.dt.float32
F32R = mybir.dt.float32r
AX = mybir.AxisListType
ALU = mybir.AluOpType
ACT = mybir.ActivationFunctionType

B = 8
T = 2048
D = 256
P = 128
KB = D // P            # 2 k-blocks of the feature dim
NT = T // P            # 16 token tiles
CH = 512               # fp32 matmul max moving free dim
NCH = T // CH          # 4 chunks
TICKS = 3
DT_STEP = 1.0
P_MIN, P_MAX = 2.0, 4.0
LN_EPS = 1e-5
TAU_EPS = 1e-6
INV_SQRT_D = 1.0 / math.sqrt(D)
UNIFORM = 1.0 / T
CONF_SCALE = 1.0 / (1.0 - UNIFORM + 1e-6)

_NC_CACHE = {}


def _build_nc():
    """Build the per-core Bass program (identical on all 8 cores)."""
    nc = bacc.Bacc("TRN2", target_bir_lowering=False, debug=False,
                   num_devices=B)

    io = {}

    def din(name, shape):
        io[name] = nc.dram_tensor(name, shape, F32,
                                  kind="ExternalInput").ap()

    for w in ("w_in", "w_q", "w_k", "w_v", "w_o", "w_p1", "w_p2"):
        din(w, [D, D])
    din("z0", [D, T])
    din("b_in_r", [1, D])
    din("b_v_r", [1, D])
    for b in ("b_q", "b_k", "b_o", "b_p1", "b_p2", "dtrt"):
        din(b, [D, 1])
    din("ident", [P, P])
    io["z_out"] = nc.dram_tensor("z_out", [D, T], F32,
                                 kind="ExternalOutput").ap()
    io["stats_out"] = nc.dram_tensor("stats", [1, 8], F32,
                                     kind="ExternalOutput").ap()

    with tile.TileContext(nc) as tc:
        _emit(nc, tc, io)
    nc.compile()
    return nc


def _emit(nc, tc, io):
    ctxs = []

    def pool(name, bufs, space="SBUF"):
        p = tc.tile_pool(name=name, bufs=bufs, space=space)
        ctxs.append(p)
        return p.__enter__()

    consts = pool("consts", 1)
    zstate = pool("zstate", 4)     # 2 tiles per generation, ping-pong
    actp = pool("actp", 1)         # hT/cT, qT, kT feature-major [128,2048]
    med = pool("med", 3)           # per-chunk scratch
    ctxsb = pool("ctxsb", 6)       # normalized ctx tiles (per-group)
    zrp = pool("zrp", 1)           # rounded f32r copy of z for matmuls
    small = pool("small", 4)       # [128,<=16] stats
    exp1p = pool("exp1p", 2)       # pass1 exp rows [128,2048]
    ptp = pool("ptp", 2)           # pass2 P_T tiles [128,512]
    # PSUM: 8 banks total, statically partitioned:
    ps_work = pool("ps_work", 2, "PSUM")   # tag work: [128,1024] -> 4 banks
    ps_ctx = pool("ps_ctx", 1, "PSUM")     # [128,4,512] -> 4 banks
    # NOTE: each matmul accumulation group must own its full 2KB PSUM
    # zero-region (start=True lazily zeroes the whole region), hence the
    # 512-f32 stride between the four ctx accumulators.

    def work(shape, name):
        return ps_work.tile(shape, F32, tag="work", name=name,
                            padded_shape=[P, 1024])

    dma = nc.sync

    # ---- constants ----
    # Matmul operands use float32r (full-rate single-pass PE fp32, ~1.5e-4
    # rel err); producers must write f32r so the HW rounds on write.
    def load_pair(name, shape):
        tiles = []
        for kb in range(KB):
            t = consts.tile(shape, F32, tag=f"{name}{kb}", name=f"{name}{kb}")
            dma.dma_start(out=t, in_=io[name][kb * P:(kb + 1) * P, :])
            tiles.append(t)
        return tiles

    def load_pair_r(name):
        tiles = []
        for kb in range(KB):
            stage = med.tile([P, D], F32, tag="wstage", name="wstage", bufs=2)
            dma.dma_start(out=stage, in_=io[name][kb * P:(kb + 1) * P, :])
            t = consts.tile([P, D], F32R, tag=f"{name}{kb}", name=f"{name}{kb}")
            nc.scalar.copy(t, stage)
            tiles.append(t)
        return tiles

    w_sb = {w: load_pair_r(w)
            for w in ("w_in", "w_q", "w_k", "w_v", "w_o", "w_p1", "w_p2")}
    col_sb = {b: load_pair(b, [P, 1])
              for b in ("b_q", "b_k", "b_o", "b_p1", "b_p2", "dtrt")}

    def load_row_r(name):
        stage = med.tile([1, D], F32, tag="rstage", name="rstage", bufs=2)
        dma.dma_start(out=stage, in_=io[name])
        t = consts.tile([1, D], F32R, tag=name, name=name)
        nc.vector.tensor_copy(t, stage)
        return t

    b_in_sb = load_row_r("b_in_r")
    b_v_sb = load_row_r("b_v_r")
    ident_sb = consts.tile([P, P], F32, tag="ident", name="ident")
    dma.dma_start(out=ident_sb, in_=io["ident"])

    ones_col = consts.tile([P, 1], F32, tag="ones_col", name="ones_col")
    nc.vector.memset(ones_col, 1.0)
    ones_row_f = consts.tile([1, P], F32, tag="ones_row_f", name="ones_row_f")
    nc.vector.memset(ones_row_f, 1.0)
    ones_row = consts.tile([1, P], F32R, tag="ones_row", name="ones_row")
    nc.vector.tensor_copy(ones_row, ones_row_f)
    stats_sb = consts.tile([1, 8], F32, tag="stats_sb", name="stats_sb")
    nc.vector.memset(stats_sb, 0.0)
    c_eps = consts.tile([P, 1], F32, tag="c_eps", name="c_eps")
    nc.vector.memset(c_eps, LN_EPS)
    c_confb = consts.tile([P, 1], F32, tag="c_confb", name="c_confb")
    nc.vector.memset(c_confb, -UNIFORM * CONF_SCALE)
    c_pmin = consts.tile([P, 1], F32, tag="c_pmin", name="c_pmin")
    nc.vector.memset(c_pmin, P_MIN)

    # V tiles persist (token-major, col D is the all-ones denominator col)
    # cols D / D+1: all-ones denominator col + pad col (fp32r matmuls
    # need an even moving-dim)
    v_sb = [consts.tile([P, D + 2], F32R, tag=f"v{i}", name=f"v{i}")
            for i in range(NT)]
    for i in range(NT):
        nc.vector.tensor_copy(v_sb[i][:, D:D + 1], ones_col)
        nc.vector.tensor_copy(v_sb[i][:, D + 1:D + 2], ones_col)

    # ---- z state (exact f32) + rounded f32r copy for matmul use ----
    z_cur = [zstate.tile([P, T], F32, tag="z", name="z") for _ in range(KB)]
    for kb in range(KB):
        dma.dma_start(out=z_cur[kb], in_=io["z0"][kb * P:(kb + 1) * P, :])

    def round_z(z_tiles):
        zr = [zrp.tile([P, T], F32R, tag=f"zr{kb}", name=f"zr{kb}")
              for kb in range(KB)]
        nc.scalar.copy(zr[0], z_tiles[0])
        nc.vector.tensor_copy(zr[1], z_tiles[1])
        return zr

    z_r = round_z(z_cur)

    for tick in range(TICKS):
        # ======== phase A: x1 (token-major) + LN + transpose to h_T ========
        h_T = [actp.tile([P, T], F32R, tag=f"hT{kb}", name=f"hT{kb}")
               for kb in range(KB)]
        for g4 in range(NT // 4):
            htoks = []
            for j in range(4):
                tt = g4 * 4 + j
                x1p = work([P, D], "x1p")
                for kb in range(KB):
                    nc.tensor.matmul(x1p, z_r[kb][:, tt * P:(tt + 1) * P],
                                     w_sb["w_in"][kb], start=(kb == 0),
                                     stop=False)
                nc.tensor.matmul(x1p, ones_row, b_in_sb, start=False,
                                 stop=True)
                x1sb = med.tile([P, D], F32, tag="x1sb", name="x1sb",
                                bufs=3)
                if tt % 2 == 0:
                    nc.scalar.copy(x1sb, x1p)
                else:
                    nc.vector.tensor_copy(x1sb, x1p)
                bn6 = small.tile([P, 6], F32, tag="bn6", name="bn6")
                nc.vector.bn_stats(bn6, x1sb)
                mv = small.tile([P, 2], F32, tag="mv", name="mv")
                nc.vector.bn_aggr(mv, bn6)
                rstd = small.tile([P, 1], F32, tag="rstd", name="rstd")
                nc.scalar.activation(rstd, mv[:, 1:2], ACT.Sqrt, bias=c_eps)
                nc.vector.reciprocal(rstd, rstd)
                htok = med.tile([P, D], F32, tag="htok", name="htok",
                                bufs=6)
                nc.vector.tensor_scalar(htok, x1sb, mv[:, 0:1], rstd,
                                        op0=ALU.subtract, op1=ALU.mult)
                htoks.append(htok)
            for db in range(KB):
                tr = work([P, CH], "trA")
                for j in range(4):
                    nc.tensor.transpose(tr[:, j * P:(j + 1) * P],
                                        htoks[j][:, db * P:(db + 1) * P],
                                        ident_sb)
                dst = h_T[db][:, g4 * CH:(g4 + 1) * CH]
                if db == 0:
                    nc.scalar.copy(dst, tr)
                else:
                    nc.vector.tensor_copy(dst, tr)

        # ======== phase B: Q, K feature-major; V token-major ========
        q_T = [actp.tile([P, T], F32R, tag=f"qT{kb}", name=f"qT{kb}")
               for kb in range(KB)]
        k_T = [actp.tile([P, T], F32R, tag=f"kT{kb}", name=f"kT{kb}")
               for kb in range(KB)]
        for db in range(KB):
            for ch in range(NCH):
                qp = work([P, CH], "qp")
                for kb in range(KB):
                    nc.tensor.matmul(qp, w_sb["w_q"][kb][:, db * P:(db + 1) * P],
                                     h_T[kb][:, ch * CH:(ch + 1) * CH],
                                     start=(kb == 0), stop=(kb == KB - 1))
                nc.scalar.activation(q_T[db][:, ch * CH:(ch + 1) * CH], qp,
                                     ACT.Identity, bias=col_sb["b_q"][db],
                                     scale=INV_SQRT_D)
                kp = work([P, CH], "kp")
                for kb in range(KB):
                    nc.tensor.matmul(kp, w_sb["w_k"][kb][:, db * P:(db + 1) * P],
                                     h_T[kb][:, ch * CH:(ch + 1) * CH],
                                     start=(kb == 0), stop=(kb == KB - 1))
                nc.vector.tensor_scalar_add(k_T[db][:, ch * CH:(ch + 1) * CH],
                                            kp, col_sb["b_k"][db])
        for st in range(NT):
            vp = work([P, D], "vp")
            for kb in range(KB):
                nc.tensor.matmul(vp, h_T[kb][:, st * P:(st + 1) * P],
                                 w_sb["w_v"][kb], start=(kb == 0), stop=False)
            nc.tensor.matmul(vp, ones_row, b_v_sb, start=False, stop=True)
            if st % 2 == 0:
                nc.scalar.copy(v_sb[st][:, 0:D], vp)
            else:
                nc.vector.tensor_copy(v_sb[st][:, 0:D], vp)

        # ======== phase C: pass 1 (plain softmax row stats) ========
        rma = small.tile([P, NT], F32, tag="rma", name="rma")
        for tt in range(NT):
            ex = exp1p.tile([P, T], F32, tag="ex", name="ex")
            ses = []
            for hf in range(2):
                sp = work([P, T // 2], "sp")
                for kb in range(KB):
                    for c2 in range(2):
                        sl = slice(c2 * CH, (c2 + 1) * CH)
                        nc.tensor.matmul(
                            sp[:, sl], q_T[kb][:, tt * P:(tt + 1) * P],
                            k_T[kb][:, hf * (T // 2) + c2 * CH:
                                    hf * (T // 2) + (c2 + 1) * CH],
                            start=(kb == 0), stop=(kb == KB - 1))
                se = small.tile([P, 1], F32, tag=f"se{hf}", name=f"se{hf}")
                nc.scalar.activation(ex[:, hf * (T // 2):(hf + 1) * (T // 2)],
                                     sp, ACT.Exp, accum_out=se)
                ses.append(se)
            sesum = small.tile([P, 1], F32, tag="sesum", name="sesum")
            nc.vector.tensor_tensor(sesum, ses[0], ses[1], op=ALU.add)
            mx = small.tile([P, 1], F32, tag="mx", name="mx")
            nc.vector.reduce_max(mx, ex, axis=AX.X)
            rcp = small.tile([P, 1], F32, tag="rcp", name="rcp")
            nc.vector.reciprocal(rcp, sesum)
            nc.vector.tensor_tensor(rma[:, tt:tt + 1], mx, rcp, op=ALU.mult)
        rms = small.tile([P, 1], F32, tag="rms", name="rms")
        nc.vector.reduce_sum(rms, rma, axis=AX.X)
        tot = work([1, 1], "tot")
        nc.tensor.matmul(tot, ones_col, rms, start=True, stop=True)
        conf = small.tile([1, 1], F32, tag="conf", name="conf")
        nc.scalar.activation(conf, tot, ACT.Identity,
                             bias=c_confb[:1], scale=CONF_SCALE / T)
        nc.vector.tensor_scalar(conf, conf, 0.0, 1.0, op0=ALU.max,
                                op1=ALU.min)
        effp = small.tile([1, 1], F32, tag="effp", name="effp")
        nc.scalar.activation(effp, conf, ACT.Identity,
                             bias=c_pmin[:1], scale=P_MAX - P_MIN)
        si = 3 + min(tick, 2)
        nc.scalar.copy(stats_sb[:, si:si + 1], effp)
        effb_p = work([P, 1], "effb")
        nc.tensor.matmul(effb_p, ones_row_f, effp, start=True, stop=True)
        effc = small.tile([P, 1], F32, tag="effc", name="effc")
        nc.scalar.copy(effc, effb_p)
        q_p2 = [actp.tile([P, T], F32R, tag=f"qp2{kb}", name=f"qp2{kb}")
                for kb in range(KB)]
        for kb in range(KB):
            nc.vector.tensor_scalar_mul(q_p2[kb], q_T[kb].bitcast(F32), effc)

        # ======== phase D: pass 2 (sharp softmax @ V) ========
        ctx_T = [actp.tile([P, T], F32R, tag=f"hT{kb}", name=f"cT{kb}")
                 for kb in range(KB)]
        for g in range(NCH):
            cx = ps_ctx.tile([P, 4, CH], F32, tag="cx", name="cx")
            for st in range(NT):
                stp = work([P, CH], "stp")
                for kb in range(KB):
                    nc.tensor.matmul(stp, k_T[kb][:, st * P:(st + 1) * P],
                                     q_p2[kb][:, g * CH:(g + 1) * CH],
                                     start=(kb == 0), stop=(kb == KB - 1))
                pt = ptp.tile([P, CH], F32R, tag="pt", name="pt")
                nc.scalar.activation(pt, stp, ACT.Exp)
                for j in range(4):
                    nc.tensor.matmul(cx[:, j, 0:D + 2],
                                     pt[:, j * P:(j + 1) * P],
                                     v_sb[st], start=(st == 0),
                                     stop=(st == NT - 1))
            gcsb = []
            for j in range(4):
                rc = small.tile([P, 1], F32, tag="rc", name="rc")
                nc.vector.reciprocal(rc, cx[:, j, D:D + 1])
                c = ctxsb.tile([P, D], F32, tag="csb", name="csb")
                nc.vector.tensor_scalar_mul(c, cx[:, j, 0:D], rc)
                gcsb.append(c)
            for db in range(KB):
                tr = work([P, CH], "trD")
                for j in range(4):
                    nc.tensor.transpose(tr[:, j * P:(j + 1) * P],
                                        gcsb[j][:, db * P:(db + 1) * P],
                                        ident_sb)
                dst = ctx_T[db][:, g * CH:(g + 1) * CH]
                if db == 0:
                    nc.scalar.copy(dst, tr)
                else:
                    nc.vector.tensor_copy(dst, tr)

        # ======== phase E: target, z update ========
        z_new = [zstate.tile([P, T], F32, tag="z", name="zn")
                 for _ in range(KB)]
        for db in range(KB):
            for ch in range(NCH):
                sl = slice(ch * CH, (ch + 1) * CH)
                tp = work([P, CH], "tp")
                for kb in range(KB):
                    nc.tensor.matmul(tp, w_sb["w_o"][kb][:, db * P:(db + 1) * P],
                                     ctx_T[kb][:, sl],
                                     start=(kb == 0), stop=(kb == KB - 1))
                dl = med.tile([P, CH], F32, tag="dl", name="dl", bufs=2)
                nc.vector.scalar_tensor_tensor(dl, tp, col_sb["b_o"][db],
                                               z_cur[db][:, sl],
                                               op0=ALU.add, op1=ALU.subtract)
                nc.vector.scalar_tensor_tensor(z_new[db][:, sl], dl,
                                               col_sb["dtrt"][db],
                                               z_cur[db][:, sl],
                                               op0=ALU.mult, op1=ALU.add)

        # ======== phase F: prediction MLP + loss ========
        lq8 = small.tile([P, 2 * NCH], F32, tag="lq8", name="lq8")
        for ch in range(NCH):
            sl = slice(ch * CH, (ch + 1) * CH)
            us = []
            for db in range(KB):
                up = work([P, CH], "up")
                for kb in range(KB):
                    nc.tensor.matmul(up, w_sb["w_p1"][kb][:, db * P:(db + 1) * P],
                                     z_r[kb][:, sl],
                                     start=(kb == 0), stop=(kb == KB - 1))
                u = med.tile([P, CH], F32R, tag=f"u{db}", name=f"u{db}",
                             bufs=2)
                nc.scalar.activation(u, up, ACT.Relu,
                                     bias=col_sb["b_p1"][db])
                us.append(u)
            for db in range(KB):
                zpp = work([P, CH], "zpp")
                for kb in range(KB):
                    nc.tensor.matmul(zpp, w_sb["w_p2"][kb][:, db * P:(db + 1) * P],
                                     us[kb], start=(kb == 0),
                                     stop=(kb == KB - 1))
                zp = med.tile([P, CH], F32, tag="zp", name="zp", bufs=2)
                nc.vector.scalar_tensor_tensor(zp, zpp, col_sb["b_p2"][db],
                                               z_new[db][:, sl],
                                               op0=ALU.add, op1=ALU.subtract)
                nc.scalar.activation(zp, zp, ACT.Square,
                                     accum_out=lq8[:, db * NCH + ch:
                                                   db * NCH + ch + 1])
        lsum = small.tile([P, 1], F32, tag="lsum", name="lsum")
        nc.vector.reduce_sum(lsum, lq8, axis=AX.X)
        ltot = work([1, 1], "ltot")
        nc.tensor.matmul(ltot, ones_col, lsum, start=True, stop=True)
        li = min(tick, 2)
        nc.scalar.copy(stats_sb[:, li:li + 1], ltot)

        z_cur = z_new
        if tick < TICKS - 1:
            z_r = round_z(z_cur)

    for kb in range(KB):
        dma.dma_start(out=io["z_out"][kb * P:(kb + 1) * P, :], in_=z_cur[kb])
    dma.dma_start(out=io["stats_out"], in_=stats_sb)

    for p in reversed(ctxs):
        p.__exit__(None, None, None)


def _host_prep(inputs):
    f = lambda k: np.asarray(inputs[k], dtype=np.float32)
    z = f("z")
    ln_g, ln_b = f("ln_g"), f("ln_b")
    cons = {}
    cons["w_in"] = np.ascontiguousarray(f("in_w").T)
    for nm in ("q", "k", "v"):
        w = f(f"{nm}_w")
        cons[f"w_{nm}"] = np.ascontiguousarray(w.T * ln_g[:, None])
    cons["w_o"] = np.ascontiguousarray(f("o_w").T)
    cons["w_p1"] = np.ascontiguousarray(f("p1_w").T)
    cons["w_p2"] = np.ascontiguousarray(f("p2_w").T)
    cons["b_in_r"] = f("in_b").reshape(1, D)
    cons["b_q"] = ((f("q_b") + f("q_w") @ ln_b) * INV_SQRT_D).reshape(D, 1)
    cons["b_k"] = (f("k_b") + f("k_w") @ ln_b).reshape(D, 1)
    cons["b_v_r"] = (f("v_b") + f("v_w") @ ln_b).reshape(1, D)
    cons["b_o"] = f("o_b").reshape(D, 1)
    cons["b_p1"] = f("p1_b").reshape(D, 1)
    cons["b_p2"] = f("p2_b").reshape(D, 1)
    cons["dtrt"] = (DT_STEP / (np.exp(f("log_tau")) + TAU_EPS)).reshape(D, 1)
    cons["ident"] = np.eye(P, dtype=np.float32)
    cons = {k: np.ascontiguousarray(v, dtype=np.float32)
            for k, v in cons.items()}
    zT = [np.ascontiguousarray(z[b].T) for b in range(B)]
    return zT, cons


def get_nc():
    if "nc" not in _NC_CACHE:
        _NC_CACHE["nc"] = _build_nc()
    return _NC_CACHE["nc"]


def make_in_maps(inputs):
    zT, cons = _host_prep(inputs)
    return [dict(cons, z0=zT[b]) for b in range(B)]


def postprocess(results):
    z_out = np.stack([results[b]["z_out"].T for b in range(B)])
    stats = np.stack([results[b]["stats"][0] for b in range(B)])  # [B, 8]
    pred_loss = np.float32(stats[:, 0:3].mean() / (T * D))
    eff_p = np.float32(stats[:, 3:6].mean())
    return np.ascontiguousarray(z_out, dtype=np.float32), pred_loss, eff_p


def kernel(**inputs):
    nc = get_nc()
    in_maps = make_in_maps(inputs)
    res = run_bass_kernel_spmd(nc, in_maps, list(range(B)))
    return postprocess(res.results)
